# revision 1
# baseline (speedup 1.0000x reference)
"""DeTPP loss kernel for 8 TRN2 NeuronCores (batch-parallel SPMD Bass/Tile).

Strategy: shard along batch B (8 per core). Per core, on device:
  - build a dense per-(l,b) record table [time-delta windows | amount
    windows | cat windows | out_time | out_amount] (DVE) and scatter it
    contiguously to a DRAM scratch table,
  - indirect-DMA row-gathers (128 rows/instr) fetch the 2048 needed
    logits rows (4KB) and record rows, interleaved on the gpsimd queue,
  - per tile: exp + per-segment sums ride the ACT accumulator; CE picks
    are fused iota-mask multiply-accumulate ops on DVE,
  - L1 terms assembled in bulk via broadcast APs; 24-permutation totals
    via one PE transpose + one block-diagonal 0/1 matmul per half;
    segmented min-reduce; masked sum and count reduced across
    partitions with PE,
  - host sums the 8 per-core (sum, count) pairs:
    loss = sum / (count * K).
"""
import sys

sys.path.insert(0, '/opt/trn_rl_repo')

import itertools
import numpy as np

L, B, I, K, C = 1024, 64, 256, 4, 256
BS = B // 8            # batch per core
R = L * BS             # rows per core (8192), row id r = l*BS + b
N = I * BS             # gathered items per core (2048)
NT = N // 128          # 16 n-tiles; tile t holds n = p*NT + t  (p = partition)
AUG = K * C + 64       # augmented row: 1024 logits + 64 rec area
RECO = K * C           # rec area offset within row
PERMS = np.array(list(itertools.permutations(range(K))), dtype=np.int32)
NP_ = PERMS.shape[0]   # 24

# rec field offsets (within 64-f32 rec area)
F_DT, F_A, F_CAT, F_OT, F_OA = 0, 4, 8, 12, 16


def _host_prep(core, time, amount, out_time, out_amount, out_cat_logits, cat,
              lengths, indices, consts):
    bsl = slice(core * BS, (core + 1) * BS)
    pad = np.zeros(64, np.float32)
    ipad = np.zeros(64, np.int32)
    return {
        "aug": np.ascontiguousarray(out_cat_logits[:, bsl]).reshape(R, K * C),
        "time_f": np.concatenate([np.ascontiguousarray(time[:, bsl]).reshape(-1), pad]),
        "amount_f": np.concatenate([np.ascontiguousarray(amount[:, bsl]).reshape(-1), pad]),
        "cat_f": np.concatenate([np.ascontiguousarray(cat[:, bsl]).reshape(-1), ipad]),
        "ot_f": np.concatenate([np.ascontiguousarray(out_time[:, bsl]).reshape(-1), pad]),
        "oa_f": np.concatenate([np.ascontiguousarray(out_amount[:, bsl]).reshape(-1), pad]),
        "idx_f": np.ascontiguousarray(indices[:, bsl]).reshape(-1),
        "len_rep": np.tile(lengths[bsl][np.arange(NT) % BS].astype(np.float32), (128, 1)),
        **consts,
    }


def _make_consts():
    pmat1 = np.zeros((K * K, NP_), np.float32)
    for p in range(NP_):
        for k in range(K):
            pmat1[k * K + PERMS[p, k], p] = 1.0
    pmat = np.zeros((128, 8 * NP_), np.float32)
    for tblk in range(8):
        pmat[tblk * 16:(tblk + 1) * 16, tblk * NP_:(tblk + 1) * NP_] = pmat1
    return {
        "iota256": np.tile(np.arange(C, dtype=np.float32), (128, 1)),
        "bpat": np.tile((np.arange(NT) % BS).astype(np.int32) * 1, (128, 1)),
        "pmat": pmat,
        "ident": np.eye(128, dtype=np.float32),
        "ones1": np.ones((128, 1), np.float32),
    }


def _build(nc, bass, mybir, tile):
    from concourse.tile_rust import add_dep_helper
    AP = bass.AP
    dt = mybir.dt
    Alu = mybir.AluOpType
    Act = mybir.ActivationFunctionType

    aug = nc.dram_tensor("aug", [R, K * C], dt.float32, kind="ExternalInput")
    rtab = nc.dram_tensor("rtab", [R, 20], dt.float32, kind="Internal")
    time_f = nc.dram_tensor("time_f", [1, R + 64], dt.float32, kind="ExternalInput")
    amount_f = nc.dram_tensor("amount_f", [1, R + 64], dt.float32, kind="ExternalInput")
    cat_f = nc.dram_tensor("cat_f", [1, R + 64], dt.int32, kind="ExternalInput")
    ot_f = nc.dram_tensor("ot_f", [1, R * K + 64], dt.float32, kind="ExternalInput")
    oa_f = nc.dram_tensor("oa_f", [1, R * K + 64], dt.float32, kind="ExternalInput")
    idx_f = nc.dram_tensor("idx_f", [1, N], dt.int32, kind="ExternalInput")
    len_rep = nc.dram_tensor("len_rep", [128, NT], dt.float32, kind="ExternalInput")
    iota256 = nc.dram_tensor("iota256", [128, C], dt.float32, kind="ExternalInput")
    bpat = nc.dram_tensor("bpat", [128, NT], dt.int32, kind="ExternalInput")
    pmat = nc.dram_tensor("pmat", [128, 8 * NP_], dt.float32, kind="ExternalInput")
    ident = nc.dram_tensor("ident", [128, 128], dt.float32, kind="ExternalInput")
    ones1 = nc.dram_tensor("ones1", [128, 1], dt.float32, kind="ExternalInput")
    out = nc.dram_tensor("out", [2, 1], dt.float32, kind="ExternalOutput")

    def dview(t, off, pattern):
        return AP(t.ap().tensor, off, pattern)

    def bc_inner(ap2, n):
        # (128, m) -> (128, m, n) broadcasting last
        a = ap2
        return AP(a.tensor, a.offset, [list(a.ap[0]), list(a.ap[1]), [0, n]])

    def bc2(ap_col, n):
        # (128, 1) -> (128, n) broadcast
        a = ap_col
        return AP(a.tensor, a.offset, [list(a.ap[0]), [0, n]])

    def bc_mid(ap2, n):
        # (128, m) -> (128, n, m) broadcasting middle
        a = ap2
        return AP(a.tensor, a.offset, [list(a.ap[0]), [0, n], list(a.ap[1])])

    with tile.TileContext(nc) as tc:
        with (
            tc.tile_pool(name="consts", bufs=1) as cpool,
            tc.tile_pool(name="rec", bufs=1) as rpool,
            tc.tile_pool(name="work", bufs=3) as wpool,
            tc.tile_pool(name="gbuf", bufs=1) as gpool,
            tc.tile_pool(name="small", bufs=1) as spool,
            tc.tile_pool(name="psum", bufs=1, space="PSUM") as ppool,
        ):
            # ---- phase B: indices
            c_len = cpool.tile([128, NT], dt.float32)
            nc.sync.dma_start(c_len[:], len_rep.ap())
            c_bpat = cpool.tile([128, NT], dt.int32)
            nc.sync.dma_start(c_bpat[:], bpat.ap())
            idxt = spool.tile([128, NT], dt.int32)
            nc.sync.dma_start(idxt[:], dview(idx_f, 0, [[NT, 128], [1, NT]]))
            idxf = spool.tile([128, NT], dt.float32)
            nc.vector.tensor_copy(out=idxf[:], in_=idxt[:])
            valid = spool.tile([128, NT], dt.float32)
            nc.vector.scalar_tensor_tensor(out=valid[:], in0=idxf[:], scalar=float(K),
                                           in1=c_len[:], op0=Alu.add, op1=Alu.is_lt)
            cnt = spool.tile([128, 1], dt.float32)
            nc.vector.tensor_reduce(out=cnt[:], in_=valid[:],
                                    axis=mybir.AxisListType.X, op=Alu.add)
            rows8 = spool.tile([128, NT], dt.int32)
            nc.vector.tensor_scalar(out=rows8[:], in0=idxt[:], scalar1=BS,
                                    scalar2=None, op0=Alu.mult)
            rowi = spool.tile([128, NT], dt.int32)
            nc.vector.tensor_tensor(out=rowi[:], in0=rows8[:], in1=c_bpat[:],
                                    op=Alu.add)
            rowf = rowi

            # ---- consts
            c_iota = cpool.tile([128, C], dt.float32)
            nc.sync.dma_start(c_iota[:], iota256.ap())
            c_pmat8 = cpool.tile([128, 8 * NP_], dt.float32)
            nc.sync.dma_start(c_pmat8[:], pmat.ap())
            c_id = cpool.tile([128, 128], dt.float32)
            nc.sync.dma_start(c_id[:], ident.ap())
            c_ones = cpool.tile([128, 1], dt.float32)
            nc.sync.dma_start(c_ones[:], ones1.ap())

            # ---- gathers: G0..G5 lead, then alternate R_t, G_{t+6}
            Gs, Rs = [None] * NT, [None] * NT

            def issue_g(t):
                G = gpool.tile([128, K * C], dt.float32, tag=f"G{t}")
                nc.gpsimd.indirect_dma_start(
                    out=G[:], out_offset=None, in_=aug.ap(),
                    in_offset=bass.IndirectOffsetOnAxis(ap=rowf[:, t:t + 1], axis=0))
                Gs[t] = G

            def issue_r(tr):
                r_ins = nc.gpsimd.indirect_dma_start(
                    out=ra[:, tr * 20:(tr + 1) * 20], out_offset=None, in_=rtab.ap(),
                    in_offset=bass.IndirectOffsetOnAxis(ap=rowf[:, tr:tr + 1], axis=0))
                add_dep_helper(r_ins.ins, scatter_ins.ins, reason="rec scatter first")
                Rs[tr] = ra[:, tr * 20:(tr + 1) * 20]

            for t in range(4):
                issue_g(t)

            # ---- phase A: rec build (dense)
            CH = R // 128  # 64 rows per partition
            OV = CH + BS * K  # overlapped load width (64 + 32)
            ttime = rpool.tile([128, OV], dt.float32, tag="ttime")
            nc.scalar.dma_start(ttime[:], dview(time_f, 0, [[CH, 128], [1, OV]]))
            tamt = rpool.tile([128, OV], dt.float32, tag="tamt")
            nc.scalar.dma_start(tamt[:], dview(amount_f, 0, [[CH, 128], [1, OV]]))
            tcat = rpool.tile([128, OV], dt.int32, tag="tcat")
            nc.scalar.dma_start(tcat[:], dview(cat_f, 0, [[CH, 128], [1, OV]]))
            t5 = [ttime[:, s * BS:s * BS + CH] for s in range(K + 1)]
            a4 = [tamt[:, s * BS:s * BS + CH] for s in range(1, K + 1)]
            c4 = [tcat[:, s * BS:s * BS + CH] for s in range(1, K + 1)]
            tot = rpool.tile([128, CH * K], dt.float32, tag="ot")
            nc.scalar.dma_start(tot[:], dview(ot_f, 0, [[CH * K, 128], [1, CH * K]]))
            toa = rpool.tile([128, CH * K], dt.float32, tag="oa")
            nc.scalar.dma_start(toa[:], dview(oa_f, 0, [[CH * K, 128], [1, CH * K]]))

            rec = rpool.tile([128, CH * 20], dt.float32, tag="rec")
            rec3 = rec[:].rearrange("p (r f) -> p r f", f=20)
            for t in range(K):
                nc.vector.tensor_tensor(out=rec3[:, :, F_DT + t], in0=t5[t + 1],
                                        in1=t5[0], op=Alu.subtract)
            for t in range(K):
                nc.vector.tensor_copy(out=rec3[:, :, F_A + t], in_=a4[t])
            for t in range(K):
                nc.vector.tensor_copy(out=rec3[:, :, F_CAT + t], in_=c4[t])
            ot3 = tot[:].rearrange("p (r f) -> p r f", f=K)
            oa3 = toa[:].rearrange("p (r f) -> p r f", f=K)
            nc.vector.tensor_copy(out=rec3[:, :, F_OT:F_OT + K], in_=ot3)
            nc.vector.tensor_copy(out=rec3[:, :, F_OA:F_OA + K], in_=oa3)

            # scatter rec to its own table (contiguous per partition)
            scatter_ins = nc.scalar.dma_start(
                dview(rtab, 0, [[CH * 20, 128], [1, CH * 20]]), rec[:])


            # ---- accumulators
            acc = spool.tile([128, NT], dt.float32)
            s4all = spool.tile([128, NT * K], dt.float32)
            costall = spool.tile([128, NT * K * K], dt.float32)
            p16all = spool.tile([128, NT * K * K], dt.float32)
            ra = spool.tile([128, NT * 20], dt.float32)

            for tr in range(NT):
                issue_r(tr)
                if 4 + tr < NT:
                    issue_g(4 + tr)

            # ---- phase C: per-tile compute
            for t in range(NT):
                G = Gs[t]
                Rg = Rs[t]

                E = wpool.tile([128, K * C], dt.float32, tag="E")
                for k in range(K):
                    nc.scalar.activation(out=E[:, k * C:(k + 1) * C],
                                         in_=G[:, k * C:(k + 1) * C], func=Act.Exp,
                                         accum_out=s4all[:, t * K + k:t * K + k + 1])

                # picks
                scr = wpool.tile([128, C], dt.float32, tag="scr")
                for k in range(K):
                    for t2 in range(K):
                        nc.vector.scalar_tensor_tensor(
                            out=scr[:], in0=c_iota[:],
                            scalar=Rg[:, F_CAT + t2:F_CAT + t2 + 1],
                            in1=G[:, k * C:(k + 1) * C],
                            op0=Alu.is_equal, op1=Alu.mult,
                            accum_out=p16all[:, t * 16 + k * K + t2:t * 16 + k * K + t2 + 1])

            # ---- bulk L1/cost assembly over all tiles
            def ra_view(field, kstep, t2step):
                a = ra[:, field:field + 1]
                return AP(a.tensor, a.offset,
                          [list(a.ap[0]), [20, NT], [kstep, K], [t2step, K]])

            d1 = spool.tile([128, NT * K * K], dt.float32)
            nc.vector.tensor_tensor(
                out=d1[:].rearrange("p (t a b) -> p t a b", a=K, b=K),
                in0=ra_view(F_OT, 1, 0), in1=ra_view(F_DT, 0, 1), op=Alu.subtract)
            nc.vector.scalar_tensor_tensor(out=d1[:], in0=d1[:], scalar=-1.0,
                                           in1=d1[:], op0=Alu.mult, op1=Alu.max)
            d2 = spool.tile([128, NT * K * K], dt.float32)
            nc.vector.tensor_tensor(
                out=d2[:].rearrange("p (t a b) -> p t a b", a=K, b=K),
                in0=ra_view(F_OA, 1, 0), in1=ra_view(F_A, 0, 1), op=Alu.subtract)
            nc.vector.scalar_tensor_tensor(out=d2[:], in0=d2[:], scalar=-1.0,
                                           in1=d2[:], op0=Alu.mult, op1=Alu.max)
            nc.vector.tensor_tensor(out=costall[:], in0=d1[:], in1=d2[:], op=Alu.add)
            nc.vector.tensor_tensor(out=costall[:], in0=costall[:], in1=p16all[:],
                                    op=Alu.subtract)

            # ---- batched PE stage: per half, 1 transpose + 1 block-diag matmul
            for h in range(2):
                pT = ppool.tile([128, 128], dt.float32, tag=f"pT{h}")
                nc.tensor.transpose(out=pT[:], in_=costall[:, h * 128:(h + 1) * 128],
                                    identity=c_id[:])
                cT = spool.tile([128, 128], dt.float32, tag=f"cT{h}")
                nc.vector.tensor_copy(out=cT[:], in_=pT[:])
                ptot = ppool.tile([128, 8 * NP_], dt.float32, tag=f"ptot{h}")
                nc.tensor.matmul(out=ptot[:], lhsT=cT[:], rhs=c_pmat8[:],
                                 start=True, stop=True)
                mint8 = wpool.tile([128, 8], dt.float32, tag=f"mint{h}")
                nc.vector.tensor_reduce(
                    out=mint8[:], in_=ptot[:].rearrange("p (t q) -> p t q", q=NP_),
                    axis=mybir.AxisListType.X, op=Alu.min)
                nc.vector.tensor_tensor(out=acc[:, h * 8:(h + 1) * 8], in0=mint8[:],
                                        in1=valid[:, h * 8:(h + 1) * 8], op=Alu.mult)

            # ---- phase D: final reduction
            lnall = spool.tile([128, NT * K], dt.float32)
            nc.scalar.activation(out=lnall[:], in_=s4all[:], func=Act.Ln)
            sall = spool.tile([128, NT], dt.float32)
            nc.vector.tensor_reduce(
                out=sall[:], in_=lnall[:].rearrange("p (t k) -> p t k", k=K),
                axis=mybir.AxisListType.X, op=Alu.add)
            nc.vector.tensor_tensor(out=sall[:], in0=sall[:], in1=valid[:],
                                    op=Alu.mult)
            nc.vector.tensor_tensor(out=acc[:], in0=acc[:], in1=sall[:], op=Alu.add)
            pair = spool.tile([128, 2], dt.float32)
            nc.vector.tensor_reduce(out=pair[:, 0:1], in_=acc[:],
                                    axis=mybir.AxisListType.X, op=Alu.add)
            nc.vector.tensor_copy(out=pair[:, 1:2], in_=cnt[:])
            pf = ppool.tile([2, 1], dt.float32, tag="pf")
            nc.tensor.matmul(out=pf[:], lhsT=pair[:], rhs=c_ones[:],
                             start=True, stop=True)
            sb = spool.tile([2, 1], dt.float32)
            nc.vector.tensor_copy(out=sb[:], in_=pf[:])
            nc.sync.dma_start(out.ap(), sb[:])
    return nc


NCORES = 8
_COMPILED = {}


def _get_compiled():
    if "nc" not in _COMPILED:
        import concourse.bacc as bacc
        import concourse.bass as bass
        import concourse.mybir as mybir
        import concourse.tile as tile
        nc = bacc.Bacc("TRN2", target_bir_lowering=False, debug=False,
                       num_devices=NCORES)
        _build(nc, bass, mybir, tile)
        nc.compile()
        _COMPILED["nc"] = nc
    return _COMPILED["nc"]


def kernel(time, amount, out_time, out_amount, out_cat_logits, cat, lengths,
           indices):
    from concourse.bass_utils import run_bass_kernel_spmd

    time = np.asarray(time, dtype=np.float32)
    amount = np.asarray(amount, dtype=np.float32)
    out_time = np.asarray(out_time, dtype=np.float32)
    out_amount = np.asarray(out_amount, dtype=np.float32)
    out_cat_logits = np.asarray(out_cat_logits, dtype=np.float32)
    cat = np.asarray(cat, dtype=np.int32)
    lengths = np.asarray(lengths, dtype=np.int32)
    indices = np.asarray(indices, dtype=np.int32)

    nc = _get_compiled()
    consts = _make_consts()
    in_maps = [
        _host_prep(c, time, amount, out_time, out_amount, out_cat_logits, cat,
                   lengths, indices, consts)
        for c in range(NCORES)
    ]
    res = run_bass_kernel_spmd(nc, in_maps, core_ids=list(range(NCORES)))
    ls = sum(float(res.results[c]["out"][0, 0]) for c in range(NCORES))
    cn = sum(float(res.results[c]["out"][1, 0]) for c in range(NCORES))
    return np.float32(ls / (cn * K))



# revision 3
# speedup vs baseline: 3.3579x; 3.3579x over previous
"""DeTPP loss kernel for 8 TRN2 NeuronCores (batch-parallel SPMD Bass/Tile).

Strategy: shard along batch B (8 per core). Host prep does only index
plumbing on tiny tensors (row ids, the K*T picked-category logits, the
L1 window fields, the valid mask: ~0.3MB/core vs the 32MB logits table).
The memory-heavy work stays on device, per core:
  - one bf16 logits table [R, K*C] in DRAM; the 2048 needed rows (2KB
    each, 4MB total) are fetched with 5 chunked indirect row-gather DMAs
    (one SWDGE descriptor per row, 16 HW DMA engines in parallel),
  - per 128-row tile: ACT exp over the full 1024-wide row, DVE segmented
    sum (bf16 2x mode) -> per-(n,k) softmax denominators; one ACT Ln at
    the end gives the logsumexp terms of the CE cost,
  - cost entries cost[n,k,t] = |ot-dt| + |oa-a| - picked_logit assembled
    with 6 bulk DVE ops on broadcast APs; the lse part (constant across
    the assignment) is added after the min,
  - 24-permutation totals via PE: transpose + block-diagonal 0/1 matmul
    per half; DVE segmented min = exact Hungarian optimum for K=4,
  - masked sum and count reduced across partitions on gpsimd; host sums
    the 8 per-core (sum, count) pairs: loss = sum / (count * K).
"""
import sys

sys.path.insert(0, '/opt/trn_rl_repo')

import itertools
import numpy as np
import ml_dtypes

BF16 = ml_dtypes.bfloat16

L, B, I, K, C = 1024, 64, 256, 4, 256
BS = B // 8            # batch per core
R = L * BS             # rows per core (8192), row id r = l*BS + b
N = I * BS             # gathered items per core (2048)
NT = N // 128          # 16 n-tiles; item n = p*NT + t  (p = partition)
PERMS = np.array(list(itertools.permutations(range(K))), dtype=np.int32)
NP_ = PERMS.shape[0]   # 24

# bigc packed-constant column layout (all bf16)
O_PICK, O_REC, O_PMAT, O_ID, O_VAL = 0, 256, 512, 704, 832
W_BIGC = 848
# rec field offsets within the 16-wide per-item rec block
F_DT, F_A, F_OT, F_OA = 0, 4, 8, 12

GATHER_CHUNKS = [(0, 2), (2, 2), (4, 4), (8, 4), (12, 4)]  # (tile start, width)


def _host_prep(core, time, amount, out_time, out_amount, out_cat_logits, cat,
               lengths, indices, consts):
    bsl = slice(core * BS, (core + 1) * BS)
    idx = indices[:, bsl].astype(np.int64)                    # (I, BS)
    bb = np.broadcast_to(np.arange(BS)[None, :], idx.shape)   # (I, BS)
    pos = (idx[:, :, None] + 1 + np.arange(K)[None, None, :]) % L  # (I,BS,K)
    bb3 = np.broadcast_to(bb[:, :, None], pos.shape)

    tloc = time[:, bsl]
    dt = tloc[pos, bb3] - tloc[idx, bb][:, :, None]           # (I, BS, K)
    aw = amount[:, bsl][pos, bb3]                             # (I, BS, K)
    cw = cat[:, bsl][pos, bb3].astype(np.int64)               # (I, BS, K)
    ot = out_time[:, bsl][idx, bb]                            # (I, BS, K)
    oa = out_amount[:, bsl][idx, bb]                          # (I, BS, K)
    ocl = out_cat_logits[:, bsl]                              # (L, BS, K, C)
    kk = np.arange(K)[None, None, :, None]
    picked = ocl[idx[:, :, None, None], bb[:, :, None, None], kk,
                 cw[:, :, None, :]]                           # (I, BS, K, T)
    valid = (idx + K < lengths[bsl].astype(np.int64)[None, :])

    bigc = np.zeros((128, W_BIGC), BF16)
    bigc[:, O_PICK:O_PICK + 256] = picked.reshape(128, 256).astype(BF16)
    rec = np.concatenate([dt, aw, ot, oa], axis=-1)           # (I, BS, 16)
    bigc[:, O_REC:O_REC + 256] = rec.reshape(128, 256).astype(BF16)
    bigc[:, O_PMAT:O_PMAT + 192] = consts["pmat"]
    bigc[:, O_ID:O_ID + 128] = consts["ident"]
    bigc[:, O_VAL:O_VAL + 16] = valid.reshape(128, 16).astype(BF16)

    rowidx = (idx * BS + bb).astype(np.int32).reshape(128, NT)
    aug = np.ascontiguousarray(out_cat_logits[:, bsl]).reshape(R, K * C)
    return {"aug": aug.astype(BF16), "bigc": bigc, "rowidx": rowidx}


def _make_consts():
    pmat1 = np.zeros((K * K, NP_), np.float32)
    for p in range(NP_):
        for k in range(K):
            pmat1[k * K + PERMS[p, k], p] = 1.0
    pmat = np.zeros((128, 8 * NP_), np.float32)
    for tblk in range(8):
        pmat[tblk * 16:(tblk + 1) * 16, tblk * NP_:(tblk + 1) * NP_] = pmat1
    return {"pmat": pmat.astype(BF16), "ident": np.eye(128, dtype=BF16)}


def _build(nc, bass, mybir, tile):
    AP = bass.AP
    dt = mybir.dt
    Alu = mybir.AluOpType
    Act = mybir.ActivationFunctionType

    aug = nc.dram_tensor("aug", [R, K * C], dt.bfloat16, kind="ExternalInput")
    bigc = nc.dram_tensor("bigc", [128, W_BIGC], dt.bfloat16,
                          kind="ExternalInput")
    rowidx = nc.dram_tensor("rowidx", [128, NT], dt.int32,
                            kind="ExternalInput")
    out = nc.dram_tensor("out", [1, 2], dt.float32, kind="ExternalOutput")

    with tile.TileContext(nc) as tc:
        with (
            tc.tile_pool(name="main", bufs=1) as pool,
            tc.tile_pool(name="psum", bufs=1, space="PSUM") as ppool,
        ):
            cb = pool.tile([128, W_BIGC], dt.bfloat16)
            nc.sync.dma_start(cb[:], bigc.ap())
            ri = pool.tile([128, NT], dt.int32)
            nc.sync.dma_start(ri[:], rowidx.ap())

            picked_v = cb[:, O_PICK:O_PICK + 256]
            pmat_v = cb[:, O_PMAT:O_PMAT + 192]
            ident_v = cb[:, O_ID:O_ID + 128]
            valid_v = cb[:, O_VAL:O_VAL + 16]

            # ---- indirect row gathers, chunked for DMA/compute overlap
            G = pool.tile([128, NT * K * C], dt.bfloat16)
            for st, w in GATHER_CHUNKS:
                nc.gpsimd.indirect_dma_start(
                    out=G[:, st * 1024:(st + w) * 1024], out_offset=None,
                    in_=aug.ap(),
                    in_offset=bass.IndirectOffsetOnAxis(ap=ri[:, st:st + w],
                                                        axis=0))

            # ---- cost16[n, k*4+t2] = |ot_k - dt_t2| + |oa_k - a_t2| - pick
            # (runs on DVE while the gathers stream)
            def rv(field, kstep, t2step):
                a = cb[:, O_REC + field:O_REC + field + 1]
                return AP(a.tensor, a.offset,
                          [list(a.ap[0]), [16, NT], [kstep, K], [t2step, K]])

            d1 = pool.tile([128, NT * K * K], dt.bfloat16)
            d13 = d1[:].rearrange("p (t a b) -> p t a b", a=K, b=K)
            nc.vector.tensor_tensor(out=d13, in0=rv(F_OT, 1, 0),
                                    in1=rv(F_DT, 0, 1), op=Alu.subtract)
            nc.vector.scalar_tensor_tensor(out=d1[:], in0=d1[:], scalar=-1.0,
                                           in1=d1[:], op0=Alu.mult,
                                           op1=Alu.max)
            d2 = pool.tile([128, NT * K * K], dt.bfloat16)
            d23 = d2[:].rearrange("p (t a b) -> p t a b", a=K, b=K)
            nc.vector.tensor_tensor(out=d23, in0=rv(F_OA, 1, 0),
                                    in1=rv(F_A, 0, 1), op=Alu.subtract)
            nc.vector.scalar_tensor_tensor(out=d2[:], in0=d2[:], scalar=-1.0,
                                           in1=d2[:], op0=Alu.mult,
                                           op1=Alu.max)
            cost16 = pool.tile([128, NT * K * K], dt.bfloat16)
            nc.vector.tensor_tensor(out=cost16[:], in0=d1[:], in1=d2[:],
                                    op=Alu.add)
            nc.vector.tensor_tensor(out=cost16[:], in0=cost16[:],
                                    in1=picked_v, op=Alu.subtract)

            # ---- 24-perm totals per half on PE, segmented min on DVE
            mint = pool.tile([128, NT], dt.float32)
            for h in range(2):
                pT = ppool.tile([128, 128], dt.bfloat16, tag=f"pT{h}")
                nc.tensor.transpose(out=pT[:],
                                    in_=cost16[:, h * 128:(h + 1) * 128],
                                    identity=ident_v)
                cT = pool.tile([128, 128], dt.bfloat16, tag=f"cT{h}")
                nc.vector.tensor_copy(out=cT[:], in_=pT[:])
                ptot = ppool.tile([128, 8 * NP_], dt.float32, tag=f"ptot{h}")
                nc.tensor.matmul(out=ptot[:], lhsT=cT[:], rhs=pmat_v,
                                 start=True, stop=True)
                nc.vector.tensor_reduce(
                    out=mint[:, h * 8:(h + 1) * 8],
                    in_=ptot[:].rearrange("p (t q) -> p t q", q=NP_),
                    axis=mybir.AxisListType.X, op=Alu.min)

            validf = pool.tile([128, NT], dt.float32)
            nc.vector.tensor_copy(out=validf[:], in_=valid_v)
            pair = pool.tile([128, 2], dt.float32)
            nc.vector.tensor_reduce(out=pair[:, 1:2], in_=validf[:],
                                    axis=mybir.AxisListType.X, op=Alu.add)

            # ---- exp + segmented sums (softmax denominators)
            E = pool.tile([128, NT * K * C], dt.bfloat16)
            s4 = pool.tile([128, NT * K], dt.bfloat16)
            for t in range(NT):
                nc.scalar.activation(out=E[:, t * 1024:(t + 1) * 1024],
                                     in_=G[:, t * 1024:(t + 1) * 1024],
                                     func=Act.Exp)
            with nc.allow_low_precision("bf16 sums; loss tolerance 2e-2"):
                for t in range(NT):
                    nc.vector.tensor_reduce(
                        out=s4[:, t * K:(t + 1) * K],
                        in_=E[:, t * 1024:(t + 1) * 1024].rearrange(
                            "p (k c) -> p k c", c=C),
                        axis=mybir.AxisListType.X, op=Alu.add)

            # ---- lse and final masked reduction
            lse = pool.tile([128, NT * K], dt.float32)
            nc.scalar.activation(out=lse[:], in_=s4[:], func=Act.Ln)
            slse = pool.tile([128, NT], dt.float32)
            nc.vector.tensor_reduce(
                out=slse[:], in_=lse[:].rearrange("p (t k) -> p t k", k=K),
                axis=mybir.AxisListType.X, op=Alu.add)
            item = pool.tile([128, NT], dt.float32)
            nc.vector.tensor_tensor(out=item[:], in0=mint[:], in1=slse[:],
                                    op=Alu.add)
            nc.vector.tensor_tensor(out=item[:], in0=item[:], in1=validf[:],
                                    op=Alu.mult)
            nc.vector.tensor_reduce(out=pair[:, 0:1], in_=item[:],
                                    axis=mybir.AxisListType.X, op=Alu.add)
            sb = pool.tile([1, 2], dt.float32)
            nc.gpsimd.tensor_reduce(out=sb[:], in_=pair[:],
                                    axis=mybir.AxisListType.C, op=Alu.add)
            nc.sync.dma_start(out.ap(), sb[:])
    return nc


NCORES = 8
_COMPILED = {}


def _get_compiled():
    if "nc" not in _COMPILED:
        import concourse.bacc as bacc
        import concourse.bass as bass
        import concourse.mybir as mybir
        import concourse.tile as tile
        nc = bacc.Bacc("TRN2", target_bir_lowering=False, debug=False,
                       num_devices=NCORES)
        _build(nc, bass, mybir, tile)
        nc.compile()
        _COMPILED["nc"] = nc
    return _COMPILED["nc"]


def kernel(time, amount, out_time, out_amount, out_cat_logits, cat, lengths,
           indices):
    from concourse.bass_utils import run_bass_kernel_spmd

    time = np.asarray(time, dtype=np.float32)
    amount = np.asarray(amount, dtype=np.float32)
    out_time = np.asarray(out_time, dtype=np.float32)
    out_amount = np.asarray(out_amount, dtype=np.float32)
    out_cat_logits = np.asarray(out_cat_logits, dtype=np.float32)
    cat = np.asarray(cat, dtype=np.int32)
    lengths = np.asarray(lengths, dtype=np.int32)
    indices = np.asarray(indices, dtype=np.int32)

    nc = _get_compiled()
    consts = _make_consts()
    in_maps = [
        _host_prep(c, time, amount, out_time, out_amount, out_cat_logits, cat,
                   lengths, indices, consts)
        for c in range(NCORES)
    ]
    res = run_bass_kernel_spmd(nc, in_maps, core_ids=list(range(NCORES)))
    ls = sum(float(res.results[c]["out"][0, 0]) for c in range(NCORES))
    cn = sum(float(res.results[c]["out"][0, 1]) for c in range(NCORES))
    return np.float32(ls / (cn * K))
